# revision 1
# baseline (speedup 1.0000x reference)
"""Trainium2 Bass kernel for nn_AttentionHead (B=4, S=2048, E=2048, DH=256).

Sharding: 8 cores = (batch b, query-half h). Each core computes attention for
1024 queries over all 2048 keys of its batch (K/V projections duplicated
across the pair of cores sharing a batch; data-parallel otherwise).

Query ownership is INTERLEAVED at 128-row block granularity (core h owns
global blocks {h, h+2, ...}) and the host passes x[b].T with key columns
permuted own-blocks-first. That makes the causal structure identical on every
core, so the SPMD program statically skips fully-masked score tiles: for its
i-th query slot a core only visits key positions {0..i} u {8..8+i}.

Scores are computed transposed (S^T[k, q]) so the softmax denominator comes
from an all-ones-matmul (no partition reductions, no P transposes); softmax
max-subtraction is skipped (scores here are bounded far below exp overflow —
test.py asserts this against the real data). All heavy matmuls run in
float32r (full PE rate at N>=256, ~TF32 precision).
"""
import sys

sys.path.insert(0, "/opt/trn_rl_repo")

import numpy as np

import concourse.bacc as bacc
import concourse.mybir as mybir
import concourse.tile as tile
from concourse.masks import make_identity

F32 = mybir.dt.float32
F32R = mybir.dt.float32r
AF = mybir.ActivationFunctionType
ALU = mybir.AluOpType

B, S, E, DH = 4, 2048, 2048, 256
SQ = S // 2          # queries per core
EC = E // 128        # contraction chunks (16)
NKT = S // 128       # key tiles (16)
CG = 512             # projection column-group width
NCG = S // CG        # projection column groups (4)
NEG = -1.0e30        # additive mask for invalid (key, query) pairs
SCALE = 1.0 / 16.0   # 1/sqrt(DH)

TRACE = False
LAST_RESULTS = None

_prog_cache = None


def core_key_order(h):
    """(qidx, kidx): global query rows owned by core h, and its key order."""
    own = np.concatenate(
        [np.arange((2 * j + h) * 128, (2 * j + h + 1) * 128) for j in range(8)]
    )
    other = np.concatenate(
        [np.arange((2 * j + 1 - h) * 128, (2 * j + 2 - h) * 128) for j in range(8)]
    )
    return own, np.concatenate([own, other])


def tiles_for_qg(qg):
    """Key-tile positions p and local query start column for query group qg."""
    out = []
    for p in range(16):
        j = p % 8
        if qg == 0:
            if j <= 3:
                out.append((p, 128 * j))
        else:
            out.append((p, 128 * max(0, j - 4)))
    return out


def _build_program():
    nc = bacc.Bacc("TRN2", target_bir_lowering=False, debug=False, num_devices=8)

    xT = nc.dram_tensor("xT", [E, S], F32R, kind="ExternalInput").ap()
    maskT = nc.dram_tensor(
        "maskT", [S, SQ], mybir.dt.bfloat16, kind="ExternalInput"
    ).ap()
    wq = nc.dram_tensor("wq", [128, EC * DH], F32R, kind="ExternalInput").ap()
    wk = nc.dram_tensor("wk", [128, EC * DH], F32R, kind="ExternalInput").ap()
    wv = nc.dram_tensor("wv", [128, EC * DH], F32R, kind="ExternalInput").ap()
    bq = nc.dram_tensor("bq", [128, 2], F32, kind="ExternalInput").ap()
    bk = nc.dram_tensor("bk", [128, 2], F32, kind="ExternalInput").ap()
    bvb = nc.dram_tensor("bvb", [128, DH], F32, kind="ExternalInput").ap()
    onesm = nc.dram_tensor("onesm", [128, 128], F32R, kind="ExternalInput").ap()
    e0 = nc.dram_tensor("e0", [128, 8], F32R, kind="ExternalInput").ap()
    out = nc.dram_tensor("out", [SQ, DH], F32, kind="ExternalOutput").ap()

    with tile.TileContext(nc) as tc:
        _emit(nc, tc, xT, maskT, wq, wk, wv, bq, bk, bvb, onesm, e0, out)
    nc.compile()
    return nc


def _emit(nc, tc, xT, maskT, wq, wk, wv, bq, bk, bvb, onesm, e0, out):
    from contextlib import ExitStack

    with ExitStack() as ctx:
        const = ctx.enter_context(tc.tile_pool(name="const", bufs=1))
        persist = ctx.enter_context(tc.tile_pool(name="persist", bufs=1))

        # ---- persistent SBUF tensors -----------------------------------
        wq_sb = const.tile([128, EC, DH], F32R, tag="wq")
        wk_sb = const.tile([128, EC, DH], F32R, tag="wk")
        wv_sb = const.tile([128, EC, DH], F32R, tag="wv")
        # weights are host-pre-tiled to [128, EC*DH] (contiguous per
        # partition) and go on the ACT HWDGE ring so they overlap the xT
        # loads on the SP ring; wk first (the first matmuls need it)
        wk_r = wk.rearrange("p (c d) -> p c d", c=EC)
        for ep in range(4):
            nc.scalar.dma_start(wk_sb[:, ep * 4:(ep + 1) * 4, :],
                                wk_r[:, ep * 4:(ep + 1) * 4, :])
        wq_r = wq.rearrange("p (c d) -> p c d", c=EC)
        for ep in range(2):
            nc.scalar.dma_start(wq_sb[:, ep * 8:(ep + 1) * 8, :],
                                wq_r[:, ep * 8:(ep + 1) * 8, :])
        wv_r = wv.rearrange("p (c d) -> p c d", c=EC)
        for ep in range(2):
            nc.scalar.dma_start(wv_sb[:, ep * 8:(ep + 1) * 8, :],
                                wv_r[:, ep * 8:(ep + 1) * 8, :])

        bq_sb = const.tile([128, 2], F32, tag="bq")
        bk_sb = const.tile([128, 2], F32, tag="bk")
        bvb_sb = const.tile([128, DH], F32, tag="bvb")
        nc.scalar.dma_start(bq_sb[:], bq[:])
        nc.scalar.dma_start(bk_sb[:], bk[:])
        nc.scalar.dma_start(bvb_sb[:], bvb[:])

        onesm_sb = const.tile([128, 128], F32R, tag="onesm")
        nc.scalar.dma_start(onesm_sb[:], onesm[:])
        e0_sb = const.tile([128, 8], F32R, tag="e0")
        nc.scalar.dma_start(e0_sb[:], e0[:])
        ident = const.tile([128, 128], F32, tag="ident")
        make_identity(nc, ident[:])

        kt_sb = persist.tile([128, 2, S], F32R, tag="ktp")
        qt_sb = persist.tile([128, 2, SQ], F32R, tag="qtp")
        v_sb = persist.tile([128, NKT, DH], F32R, tag="vp")
        ot_sb = persist.tile([128, 2, SQ], F32, tag="otp")
        dn_sb = persist.tile([128, SQ], F32R, tag="dnp")

        # ---- phase P: projections --------------------------------------
        xT_r = xT.rearrange("(c p) s -> p c s", p=128)
        with tc.tile_pool(name="xt", bufs=2) as xt_pool, \
             tc.tile_pool(name="proj_ps", bufs=2, space="PSUM") as proj_ps, \
             tc.tile_pool(name="v_ps", bufs=2, space="PSUM") as v_ps:
            for cg in range(NCG):
                xt = xt_pool.tile([128, EC, CG], F32R, tag="xt")
                npc = 4 if cg == 0 else 2
                w = EC // npc
                for ep in range(npc):
                    nc.sync.dma_start(
                        xt[:, ep * w:(ep + 1) * w, :],
                        xT_r[:, ep * w:(ep + 1) * w, cg * CG:(cg + 1) * CG],
                    )
                for dh2 in range(2):
                    ps = proj_ps.tile([128, CG], F32, tag="proj")
                    for e in range(EC):
                        nc.tensor.matmul(
                            ps[:],
                            wk_sb[:, e, dh2 * 128:(dh2 + 1) * 128],
                            xt[:, e, :],
                            start=(e == 0),
                            stop=(e == EC - 1),
                        )
                    nc.scalar.activation(
                        kt_sb[:, dh2, cg * CG:(cg + 1) * CG], ps[:],
                        AF.Identity, bias=bk_sb[:, dh2:dh2 + 1],
                    )
                if cg * CG < SQ:
                    for dh2 in range(2):
                        ps = proj_ps.tile([128, CG], F32, tag="proj")
                        for e in range(EC):
                            nc.tensor.matmul(
                                ps[:],
                                wq_sb[:, e, dh2 * 128:(dh2 + 1) * 128],
                                xt[:, e, :],
                                start=(e == 0),
                                stop=(e == EC - 1),
                            )
                        nc.scalar.activation(
                            qt_sb[:, dh2, cg * CG:(cg + 1) * CG], ps[:],
                            AF.Identity, bias=bq_sb[:, dh2:dh2 + 1],
                        )
                for kt4 in range(CG // 128):
                    kt = cg * (CG // 128) + kt4
                    psv = v_ps.tile([128, DH], F32, tag="vps")
                    for e in range(EC):
                        nc.tensor.matmul(
                            psv[:],
                            xt[:, e, kt4 * 128:(kt4 + 1) * 128],
                            wv_sb[:, e, :],
                            start=(e == 0),
                            stop=(e == EC - 1),
                        )
                    nc.vector.tensor_copy(v_sb[:, kt, :], psv[:])

        # ---- phase A: attention + per-group finalize -------------------
        with tc.tile_pool(name="s_ps", bufs=3, space="PSUM") as s_ps, \
             tc.tile_pool(name="o_ps", bufs=1, space="PSUM") as o_ps, \
             tc.tile_pool(name="d_ps", bufs=1, space="PSUM") as d_ps, \
             tc.tile_pool(name="f_ps", bufs=1, space="PSUM") as f_ps, \
             tc.tile_pool(name="r_ps", bufs=1, space="PSUM") as r_ps, \
             tc.tile_pool(name="mk", bufs=12) as mk_pool, \
             tc.tile_pool(name="pt", bufs=3) as pt_pool, \
             tc.tile_pool(name="fin", bufs=3) as fin:
            for qg in range(2):
                q0 = qg * 512
                tiles = tiles_for_qg(qg)
                otp = [
                    o_ps.tile([128, 512], F32, tag=f"ot{d}", name=f"otp{qg}_{d}")
                    for d in range(2)
                ]
                dnp = d_ps.tile([128, 512], F32, tag="dn")
                for ti, (p, qs) in enumerate(tiles):
                    n = 512 - qs
                    sp = s_ps.tile([128, 512], F32, tag="sp")
                    for dh2 in range(2):
                        nc.tensor.matmul(
                            sp[:, :n],
                            kt_sb[:, dh2, p * 128:(p + 1) * 128],
                            qt_sb[:, dh2, q0 + qs:q0 + 512],
                            start=(dh2 == 0),
                            stop=(dh2 == 1),
                        )
                    mk = mk_pool.tile([128, 512], mybir.dt.bfloat16, tag="mk")
                    nc.sync.dma_start(
                        mk[:, :n], maskT[p * 128:(p + 1) * 128, q0 + qs:q0 + 512]
                    )
                    nc.vector.tensor_tensor(
                        sp[:, :n], sp[:, :n], mk[:, :n], op=ALU.add
                    )
                    pt = pt_pool.tile([128, 512], F32R, tag="pt")
                    nc.scalar.activation(pt[:, :n], sp[:, :n], AF.Exp, scale=SCALE)
                    for dh2 in range(2):
                        nc.tensor.matmul(
                            otp[dh2][:, qs:512],
                            v_sb[:, p, dh2 * 128:(dh2 + 1) * 128],
                            pt[:, :n],
                            start=(ti == 0),
                            stop=(ti == len(tiles) - 1),
                        )
                    nc.tensor.matmul(
                        dnp[:, qs:512],
                        onesm_sb[:],
                        pt[:, :n],
                        start=(ti == 0),
                        stop=(ti == len(tiles) - 1),
                    )
                for dh2 in range(2):
                    nc.vector.tensor_copy(ot_sb[:, dh2, q0:q0 + 512], otp[dh2][:])
                nc.vector.tensor_copy(dn_sb[:, q0:q0 + 512], dnp[:])

                # finalize this query group: recip + transpose + scale + store
                rtp = r_ps.tile([128, 32], F32, tag="rt", name=f"rtp{qg}")
                for qi in range(4):
                    qt = qg * 4 + qi
                    nc.tensor.matmul(
                        rtp[:, qi * 8:(qi + 1) * 8],
                        dn_sb[:, qt * 128:(qt + 1) * 128],
                        e0_sb[:],
                        start=(qi == 0),
                        stop=(qi == 3),
                    )
                rt_sb = fin.tile([128, 32], F32, tag="rtsb")
                nc.vector.tensor_copy(rt_sb[:], rtp[:])
                rc_sb = fin.tile([128, 32], F32, tag="rcsb")
                nc.vector.reciprocal(rc_sb[:], rt_sb[:])
                for qi in range(4):
                    qt = qg * 4 + qi
                    ofp = f_ps.tile([128, DH], F32, tag="of")
                    for dh2 in range(2):
                        nc.tensor.transpose(
                            ofp[:, dh2 * 128:(dh2 + 1) * 128],
                            ot_sb[:, dh2, qt * 128:(qt + 1) * 128],
                            ident[:],
                        )
                    ob = fin.tile([128, DH], F32, tag="ob")
                    nc.scalar.mul(ob[:], ofp[:], rc_sb[:, qi * 8:qi * 8 + 1])
                    nc.vector.tensor_tensor(ob[:], ob[:], bvb_sb[:], op=ALU.add)
                    nc.scalar.dma_start(out[qt * 128:(qt + 1) * 128, :], ob[:])


def _get_program():
    global _prog_cache
    if _prog_cache is None:
        _prog_cache = _build_program()
    return _prog_cache


def kernel(x, causal_mask, padding_mask, W_Q, b_Q, W_K, b_K, W_V, b_V):
    global LAST_RESULTS
    from concourse.bass_utils import run_bass_kernel_spmd

    import ml_dtypes

    x = np.ascontiguousarray(x, dtype=np.float32)
    causal = np.asarray(causal_mask) != 0            # [S, S] attend where True
    pad = np.asarray(padding_mask)                   # [B, S]  True = masked key

    def tile_w(W):
        W = np.asarray(W, dtype=np.float32)
        return np.ascontiguousarray(
            W.reshape(EC, 128, DH).transpose(1, 0, 2).reshape(128, EC * DH)
        )

    W_Q = tile_w(W_Q)
    W_K = tile_w(W_K)
    W_V = tile_w(W_V)
    bvb = np.ascontiguousarray(
        np.broadcast_to(np.asarray(b_V, dtype=np.float32), (128, DH))
    )
    bq = np.ascontiguousarray(
        np.asarray(b_Q, dtype=np.float32).reshape(2, 128).T
    )
    bk = np.ascontiguousarray(
        np.asarray(b_K, dtype=np.float32).reshape(2, 128).T
    )
    e0v = np.zeros((128, 8), dtype=np.float32)
    e0v[0, :] = 1.0
    onesm = np.ones((128, 128), dtype=np.float32)

    in_maps = []
    for c in range(8):
        b, h = c // 2, c % 2
        qidx, kidx = core_key_order(h)
        xT = np.ascontiguousarray(x[b][kidx].T)      # [E, S] keys permuted
        valid = causal[np.ix_(qidx, kidx)].T & ~pad[b][kidx][:, None]
        mT = np.where(valid, np.float32(0.0), np.float32(NEG))
        in_maps.append({
            "xT": xT,
            "maskT": np.ascontiguousarray(mT.astype(ml_dtypes.bfloat16)),
            "wq": W_Q, "wk": W_K, "wv": W_V,
            "bq": bq, "bk": bk, "bvb": bvb,
            "onesm": onesm, "e0": e0v,
        })

    nc = _get_program()
    res = run_bass_kernel_spmd(nc, in_maps, list(range(8)), trace=TRACE)
    LAST_RESULTS = res

    outp = np.empty((B, S, DH), dtype=np.float32)
    for c in range(8):
        b, h = c // 2, c % 2
        qidx, _ = core_key_order(h)
        outp[b][qidx] = res.results[c]["out"]
    return outp



# revision 4
# speedup vs baseline: 1.2208x; 1.2208x over previous
"""Trainium2 Bass kernel for nn_AttentionHead (B=4, S=2048, E=2048, DH=256).

Sharding: 8 cores = (batch b, query-half h). Each core computes attention for
1024 queries over all 2048 keys of its batch (K/V projections duplicated
across the pair of cores sharing a batch; data-parallel otherwise).

Query ownership is INTERLEAVED at 128-row block granularity (core h owns
global blocks {h, h+2, ...}) and the host passes x[b].T with key columns
permuted own-blocks-first. That makes the causal structure near-identical on
every core, so the SPMD program statically skips fully-masked score tiles.
The residual h-asymmetry (other-core key block j is valid from query tile
j+1 on h=0 but from j on h=1) plus key-padding is folded into a tiny
per-exp-instruction bias table ([128, NI] f32): softmax masking costs no
score-sized DMA at all. The causal triangle on diagonal tiles is a single
[128,128] additive constant.

All heavy matmuls run in bf16 (full PE rate at any tile size); accumulation
stays fp32 in PSUM. Scores are computed transposed (S^T[k, q]) so the softmax
denominator comes from an all-ones matmul; softmax max-subtraction is skipped
(scores are bounded far below fp32 exp overflow).

Attention runs in 4 query groups of 256 so the final-group epilogue
(reciprocal + transpose + scale + store) is short.
"""
import sys

sys.path.insert(0, "/opt/trn_rl_repo")

import numpy as np

import concourse.bacc as bacc
import concourse.mybir as mybir
import concourse.tile as tile

F32 = mybir.dt.float32
F32R = mybir.dt.float32r
BF16 = mybir.dt.bfloat16
AF = mybir.ActivationFunctionType
ALU = mybir.AluOpType

B, S, E, DH = 4, 2048, 2048, 256
SQ = S // 2          # queries per core
EC = E // 128        # contraction chunks (16)
CG = 512             # projection column-group width
NCG = S // CG        # projection column groups (4)
GQ = 256             # attention query-group width
NG = SQ // GQ        # attention query groups (4)
NEG = -1.0e5         # additive mask (exp(-1e5/16) == 0 in f32)
SCALE = 1.0 / 16.0   # 1/sqrt(DH)

TRACE = False
LAST_RESULTS = None

_prog_cache = None


def core_key_order(h):
    """(qidx, kidx): global query rows owned by core h, and its key order."""
    own = np.concatenate(
        [np.arange((2 * j + h) * 128, (2 * j + h + 1) * 128) for j in range(8)]
    )
    other = np.concatenate(
        [np.arange((2 * j + 1 - h) * 128, (2 * j + 2 - h) * 128) for j in range(8)]
    )
    return own, np.concatenate([own, other])


def group_tiles(g):
    """Score tiles for query group g (local query tiles t0=2g, t1=2g+1).

    Returns a list of (p, qs, n, tri, segs): key-tile position p, group-column
    start qs, width n, whether the causal triangle applies (always at local
    cols 0..128 when True), and exp segments [(group_colstart, width, kind)]
    with kind 'pad' (key padding only) or 'kill' (whole block masked on h=0,
    padding-only on h=1).
    """
    t0, t1 = 2 * g, 2 * g + 1
    out = []
    for p in range(8):            # own keys: valid for t >= p, diagonal at t==p
        if p < t0:
            out.append((p, 0, 256, False, [(0, 256, "pad")]))
        elif p == t0:
            out.append((p, 0, 256, True, [(0, 256, "pad")]))
        elif p == t1:
            out.append((p, 128, 128, True, [(128, 128, "pad")]))
    for j in range(8):            # other keys: valid for t >= j+1 (h=0) / t >= j (h=1)
        p = 8 + j
        if j < t0:
            out.append((p, 0, 256, False, [(0, 256, "pad")]))
        elif j == t0:
            out.append((p, 0, 256, False, [(0, 128, "kill"), (128, 128, "pad")]))
        elif j == t1:
            out.append((p, 128, 128, False, [(128, 128, "kill")]))
    return out


def bias_columns():
    """Flat (p, kind) per bias-table column, in device emit order."""
    cols = []
    for g in range(NG):
        for p, qs, n, tri, segs in group_tiles(g):
            for cs, w, kind in segs:
                cols.append((p, kind))
    return cols


NI = len(bias_columns())


def _build_program():
    nc = bacc.Bacc("TRN2", target_bir_lowering=False, debug=False, num_devices=8)

    xT = nc.dram_tensor("xT", [E, S], BF16, kind="ExternalInput").ap()
    wq = nc.dram_tensor("wq", [128, EC * DH], BF16, kind="ExternalInput").ap()
    wk = nc.dram_tensor("wk", [128, EC * DH], BF16, kind="ExternalInput").ap()
    wv = nc.dram_tensor("wv", [128, EC * DH], BF16, kind="ExternalInput").ap()
    bq = nc.dram_tensor("bq", [128, 2], F32, kind="ExternalInput").ap()
    bk = nc.dram_tensor("bk", [128, 2], F32, kind="ExternalInput").ap()
    bvb = nc.dram_tensor("bvb", [128, DH], F32, kind="ExternalInput").ap()
    onesm = nc.dram_tensor("onesm", [128, 128], BF16, kind="ExternalInput").ap()
    ident = nc.dram_tensor("ident", [128, 128], BF16, kind="ExternalInput").ap()
    trim = nc.dram_tensor("trim", [128, 128], F32, kind="ExternalInput").ap()
    btab = nc.dram_tensor("btab", [128, NI], F32, kind="ExternalInput").ap()
    e0 = nc.dram_tensor("e0", [128, 8], F32R, kind="ExternalInput").ap()
    out = nc.dram_tensor("out", [SQ, DH], F32, kind="ExternalOutput").ap()

    with tile.TileContext(nc) as tc:
        _emit(nc, tc, xT, wq, wk, wv, bq, bk, bvb, onesm, ident, trim, btab,
              e0, out)
    nc.compile()
    return nc


def _emit(nc, tc, xT, wq, wk, wv, bq, bk, bvb, onesm, ident, trim, btab, e0,
          out):
    from contextlib import ExitStack

    with ExitStack() as ctx:
        const = ctx.enter_context(tc.tile_pool(name="const", bufs=1))
        persist = ctx.enter_context(tc.tile_pool(name="persist", bufs=1))

        # ---- persistent SBUF tensors -----------------------------------
        wq_sb = const.tile([128, EC, DH], BF16, tag="wq")
        wk_sb = const.tile([128, EC, DH], BF16, tag="wk")
        wv_sb = const.tile([128, EC, DH], BF16, tag="wv")
        # weights go on the ACT HWDGE ring (overlapping the xT loads on the
        # SP ring); wk first, in 4 chunks, so the first matmuls start early
        wk_r = wk.rearrange("p (c d) -> p c d", c=EC)
        for ep in range(4):
            nc.scalar.dma_start(wk_sb[:, ep * 4:(ep + 1) * 4, :],
                                wk_r[:, ep * 4:(ep + 1) * 4, :])
        wq_r = wq.rearrange("p (c d) -> p c d", c=EC)
        for ep in range(2):
            nc.scalar.dma_start(wq_sb[:, ep * 8:(ep + 1) * 8, :],
                                wq_r[:, ep * 8:(ep + 1) * 8, :])
        wv_r = wv.rearrange("p (c d) -> p c d", c=EC)
        for ep in range(2):
            nc.scalar.dma_start(wv_sb[:, ep * 8:(ep + 1) * 8, :],
                                wv_r[:, ep * 8:(ep + 1) * 8, :])

        # small constants on the gpsimd SWDGE ring: zero interference
        bq_sb = const.tile([128, 2], F32, tag="bq")
        bk_sb = const.tile([128, 2], F32, tag="bk")
        bvb_sb = const.tile([128, DH], F32, tag="bvb")
        onesm_sb = const.tile([128, 128], BF16, tag="onesm")
        ident_sb = const.tile([128, 128], BF16, tag="ident")
        trim_sb = const.tile([128, 128], F32, tag="trim")
        btab_sb = const.tile([128, NI], F32, tag="btab")
        e0_sb = const.tile([128, 8], F32R, tag="e0")
        nc.gpsimd.dma_start(bk_sb[:], bk[:])
        nc.gpsimd.dma_start(bq_sb[:], bq[:])
        nc.gpsimd.dma_start(btab_sb[:], btab[:])
        nc.gpsimd.dma_start(trim_sb[:], trim[:])
        nc.gpsimd.dma_start(onesm_sb[:], onesm[:])
        nc.gpsimd.dma_start(ident_sb[:], ident[:])
        nc.gpsimd.dma_start(e0_sb[:], e0[:])
        nc.gpsimd.dma_start(bvb_sb[:], bvb[:])

        kt_sb = persist.tile([128, 2, S], BF16, tag="ktp")
        qt_sb = persist.tile([128, 2, SQ], BF16, tag="qtp")
        v_sb = persist.tile([128, S // 128, DH], BF16, tag="vp")

        # ---- phase P: projections --------------------------------------
        xT_r = xT.rearrange("(c p) s -> p c s", p=128)
        with tc.tile_pool(name="xt", bufs=2) as xt_pool, \
             tc.tile_pool(name="proj_ps", bufs=2, space="PSUM") as proj_ps, \
             tc.tile_pool(name="v_ps", bufs=2, space="PSUM") as v_ps:
            for cg in range(NCG):
                xt = xt_pool.tile([128, EC, CG], BF16, tag="xt")
                npc = 4 if cg == 0 else 2
                w = EC // npc
                for ep in range(npc):
                    nc.sync.dma_start(
                        xt[:, ep * w:(ep + 1) * w, :],
                        xT_r[:, ep * w:(ep + 1) * w, cg * CG:(cg + 1) * CG],
                    )
                for dh2 in range(2):
                    ps = proj_ps.tile([128, CG], F32, tag="proj")
                    for e in range(EC):
                        nc.tensor.matmul(
                            ps[:],
                            wk_sb[:, e, dh2 * 128:(dh2 + 1) * 128],
                            xt[:, e, :],
                            start=(e == 0),
                            stop=(e == EC - 1),
                        )
                    nc.scalar.activation(
                        kt_sb[:, dh2, cg * CG:(cg + 1) * CG], ps[:],
                        AF.Identity, bias=bk_sb[:, dh2:dh2 + 1],
                    )
                if cg * CG < SQ:
                    for dh2 in range(2):
                        ps = proj_ps.tile([128, CG], F32, tag="proj")
                        for e in range(EC):
                            nc.tensor.matmul(
                                ps[:],
                                wq_sb[:, e, dh2 * 128:(dh2 + 1) * 128],
                                xt[:, e, :],
                                start=(e == 0),
                                stop=(e == EC - 1),
                            )
                        nc.scalar.activation(
                            qt_sb[:, dh2, cg * CG:(cg + 1) * CG], ps[:],
                            AF.Identity, bias=bq_sb[:, dh2:dh2 + 1],
                        )
                for kt4 in range(CG // 128):
                    kt = cg * (CG // 128) + kt4
                    psv = v_ps.tile([128, DH], F32, tag="vps")
                    for e in range(EC):
                        nc.tensor.matmul(
                            psv[:],
                            xt[:, e, kt4 * 128:(kt4 + 1) * 128],
                            wv_sb[:, e, :],
                            start=(e == 0),
                            stop=(e == EC - 1),
                        )
                    nc.vector.tensor_copy(v_sb[:, kt, :], psv[:])

        # ---- phase A: attention + per-group finalize -------------------
        bi = 0  # running bias-table column index (must match bias_columns())
        # PSUM budget: slots round up to whole banks, 8 total:
        # s_ps 3 + o_ps 2 (two tags) + d_ps 1 + f_ps 1 + r_ps 1 = 8
        with tc.tile_pool(name="s_ps", bufs=3, space="PSUM") as s_ps, \
             tc.tile_pool(name="o_ps", bufs=1, space="PSUM") as o_ps, \
             tc.tile_pool(name="d_ps", bufs=1, space="PSUM") as d_ps, \
             tc.tile_pool(name="f_ps", bufs=1, space="PSUM") as f_ps, \
             tc.tile_pool(name="r_ps", bufs=1, space="PSUM") as r_ps, \
             tc.tile_pool(name="pt", bufs=3) as pt_pool, \
             tc.tile_pool(name="fin", bufs=2) as fin:
            for g in range(NG):
                q0 = g * GQ
                tiles = group_tiles(g)
                otp = [
                    o_ps.tile([128, GQ], F32, tag=f"ot{d}", name=f"otp{g}_{d}")
                    for d in range(2)
                ]
                dnp = d_ps.tile([128, GQ], F32, tag="dn")
                last = len(tiles) - 1
                for ti, (p, qs, n, tri, segs) in enumerate(tiles):
                    sp = s_ps.tile([128, GQ], F32, tag="sp")
                    for dh2 in range(2):
                        nc.tensor.matmul(
                            sp[:, :n],
                            kt_sb[:, dh2, p * 128:(p + 1) * 128],
                            qt_sb[:, dh2, q0 + qs:q0 + qs + n],
                            start=(dh2 == 0),
                            stop=(dh2 == 1),
                        )
                    if tri:
                        nc.vector.tensor_tensor(
                            sp[:, :128], sp[:, :128], trim_sb[:], op=ALU.add
                        )
                    pt = pt_pool.tile([128, GQ], BF16, tag="pt")
                    for cs, w, kind in segs:
                        nc.scalar.activation(
                            pt[:, cs - qs:cs - qs + w],
                            sp[:, cs - qs:cs - qs + w],
                            AF.Exp, scale=SCALE, bias=btab_sb[:, bi:bi + 1],
                        )
                        bi += 1
                    for dh2 in range(2):
                        nc.tensor.matmul(
                            otp[dh2][:, qs:qs + n],
                            v_sb[:, p, dh2 * 128:(dh2 + 1) * 128],
                            pt[:, :n],
                            start=(ti == 0),
                            stop=(ti == last),
                        )
                    nc.tensor.matmul(
                        dnp[:, qs:qs + n],
                        onesm_sb[:],
                        pt[:, :n],
                        start=(ti == 0),
                        stop=(ti == last),
                    )

                # finalize this query group: recip + transpose + scale + store
                ot_g = fin.tile([128, 2, GQ], BF16, tag="otg")
                for dh2 in range(2):
                    nc.vector.tensor_copy(ot_g[:, dh2, :], otp[dh2][:])
                dn_g = fin.tile([128, GQ], F32R, tag="dng")
                nc.vector.tensor_copy(dn_g[:], dnp[:])
                rtp = r_ps.tile([128, 16], F32, tag="rt", name=f"rtp{g}")
                for qi in range(2):
                    nc.tensor.matmul(
                        rtp[:, qi * 8:(qi + 1) * 8],
                        dn_g[:, qi * 128:(qi + 1) * 128],
                        e0_sb[:],
                        start=(qi == 0),
                        stop=(qi == 1),
                    )
                rt_sb = fin.tile([128, 16], F32, tag="rtsb")
                nc.vector.tensor_copy(rt_sb[:], rtp[:])
                rc_sb = fin.tile([128, 16], F32, tag="rcsb")
                nc.vector.reciprocal(rc_sb[:], rt_sb[:])
                for qi in range(2):
                    qt = g * 2 + qi
                    ofp = f_ps.tile([128, DH], BF16, tag="of")
                    for dh2 in range(2):
                        nc.tensor.transpose(
                            ofp[:, dh2 * 128:(dh2 + 1) * 128],
                            ot_g[:, dh2, qi * 128:(qi + 1) * 128],
                            ident_sb[:],
                        )
                    ob = fin.tile([128, DH], F32, tag="ob")
                    nc.scalar.mul(ob[:], ofp[:], rc_sb[:, qi * 8:qi * 8 + 1])
                    nc.vector.tensor_tensor(ob[:], ob[:], bvb_sb[:], op=ALU.add)
                    nc.scalar.dma_start(out[qt * 128:(qt + 1) * 128, :], ob[:])


def _get_program():
    global _prog_cache
    if _prog_cache is None:
        _prog_cache = _build_program()
    return _prog_cache


def kernel(x, causal_mask, padding_mask, W_Q, b_Q, W_K, b_K, W_V, b_V):
    global LAST_RESULTS
    from concourse.bass_utils import run_bass_kernel_spmd

    import ml_dtypes

    bf16 = ml_dtypes.bfloat16
    x = np.asarray(x, dtype=np.float32)
    pad = np.asarray(padding_mask)                   # [B, S]  True = masked key

    def tile_w(W):
        W = np.asarray(W, dtype=np.float32)
        return np.ascontiguousarray(
            W.reshape(EC, 128, DH).transpose(1, 0, 2).reshape(128, EC * DH)
        ).astype(bf16)

    W_Qb = tile_w(W_Q)
    W_Kb = tile_w(W_K)
    W_Vb = tile_w(W_V)
    bvb = np.ascontiguousarray(
        np.broadcast_to(np.asarray(b_V, dtype=np.float32), (128, DH))
    )
    bqh = np.ascontiguousarray(np.asarray(b_Q, dtype=np.float32).reshape(2, 128).T)
    bkh = np.ascontiguousarray(np.asarray(b_K, dtype=np.float32).reshape(2, 128).T)
    e0v = np.zeros((128, 8), dtype=np.float32)
    e0v[0, :] = 1.0
    onesm = np.ones((128, 128), dtype=bf16)
    identm = np.eye(128, dtype=np.float32).astype(bf16)
    q = np.arange(128)
    trimv = np.where(q[None, :] >= q[:, None], np.float32(0.0),
                     np.float32(NEG))  # [k, q]: attend iff q >= k
    trimv = np.ascontiguousarray(trimv)

    cols = bias_columns()
    in_maps = []
    for c in range(8):
        b, h = c // 2, c % 2
        qidx, kidx = core_key_order(h)
        xT = np.ascontiguousarray(x[b][kidx].T).astype(bf16)   # [E, S] permuted
        padp = pad[b][kidx].reshape(16, 128)                   # [tile, row]
        bt = np.zeros((128, len(cols)), dtype=np.float32)
        for i, (p, kind) in enumerate(cols):
            col = np.where(padp[p], np.float32(NEG), np.float32(0.0))
            if kind == "kill" and h == 0:
                col = np.full(128, np.float32(NEG))
            bt[:, i] = col
        in_maps.append({
            "xT": xT,
            "wq": W_Qb, "wk": W_Kb, "wv": W_Vb,
            "bq": bqh, "bk": bkh, "bvb": bvb,
            "onesm": onesm, "ident": identm, "trim": trimv,
            "btab": np.ascontiguousarray(bt), "e0": e0v,
        })

    nc = _get_program()
    res = run_bass_kernel_spmd(nc, in_maps, list(range(8)), trace=TRACE)
    LAST_RESULTS = res

    outp = np.empty((B, S, DH), dtype=np.float32)
    for c in range(8):
        b, h = c // 2, c % 2
        qidx, _ = core_key_order(h)
        outp[b][qidx] = res.results[c]["out"]
    return outp


# revision 8
# speedup vs baseline: 1.2583x; 1.0307x over previous
"""Trainium2 Bass kernel for nn_AttentionHead (B=4, S=2048, E=2048, DH=256).

Sharding: 8 cores = (batch b, query-half h). Each core computes attention for
1024 queries over all 2048 keys of its batch (K/V projections duplicated
across the pair of cores sharing a batch; data-parallel otherwise).

Query ownership is INTERLEAVED at 128-row block granularity (core h owns
global blocks {h, h+2, ...}) and the host passes x[b].T with key columns
permuted own-blocks-first. That makes the causal structure near-identical on
every core, so the SPMD program statically skips fully-masked score tiles.
The residual h-asymmetry (other-core key block j is valid from query tile
j+1 on h=0 but from j on h=1) plus key-padding is folded into a tiny
per-exp-instruction bias table ([128, NI] f32): softmax masking costs no
score-sized DMA at all. The causal triangle on diagonal tiles is a single
[128,128] additive constant.

All heavy matmuls run in bf16 (full PE rate at any tile size); accumulation
stays fp32 in PSUM. Scores are computed transposed (S^T[k, q]) so the softmax
denominator comes from an all-ones matmul; softmax max-subtraction is skipped
(scores are bounded far below fp32 exp overflow).

Attention runs in 4 query groups of 256 so the final-group epilogue
(reciprocal + transpose + scale + store) is short.
"""
import sys

sys.path.insert(0, "/opt/trn_rl_repo")

import numpy as np

import concourse.bacc as bacc
import concourse.mybir as mybir
import concourse.tile as tile

F32 = mybir.dt.float32
F32R = mybir.dt.float32r
BF16 = mybir.dt.bfloat16
AF = mybir.ActivationFunctionType
ALU = mybir.AluOpType

B, S, E, DH = 4, 2048, 2048, 256
SQ = S // 2          # queries per core
EC = E // 128        # contraction chunks (16)
CG = 512             # projection column-group width
NCG = S // CG        # projection column groups (4)
GQ = 256             # attention query-group width
NG = SQ // GQ        # attention query groups (4)
NEG = -1.0e5         # additive mask (exp(-1e5/16) == 0 in f32)
SCALE = 1.0 / 16.0   # 1/sqrt(DH)

TRACE = False
LAST_RESULTS = None

_prog_cache = None


def core_key_order(h):
    """(qidx, kidx): global query rows owned by core h, and its key order."""
    own = np.concatenate(
        [np.arange((2 * j + h) * 128, (2 * j + h + 1) * 128) for j in range(8)]
    )
    other = np.concatenate(
        [np.arange((2 * j + 1 - h) * 128, (2 * j + 2 - h) * 128) for j in range(8)]
    )
    return own, np.concatenate([own, other])


def group_tiles(g):
    """Score tiles for query group g (local query tiles t0=2g, t1=2g+1).

    Returns a list of (p, qs, n, tri, segs): key-tile position p, group-column
    start qs, width n, whether the causal triangle applies (always at local
    cols 0..128 when True), and exp segments [(group_colstart, width, kind)]
    with kind 'pad' (key padding only) or 'kill' (whole block masked on h=0,
    padding-only on h=1).
    """
    t0, t1 = 2 * g, 2 * g + 1
    out = []
    for p in range(8):            # own keys: valid for t >= p, diagonal at t==p
        if p < t0:
            out.append((p, 0, 256, False, [(0, 256, "pad")]))
        elif p == t0:
            out.append((p, 0, 256, True, [(0, 256, "pad")]))
        elif p == t1:
            out.append((p, 128, 128, True, [(128, 128, "pad")]))
    for j in range(8):            # other keys: valid for t >= j+1 (h=0) / t >= j (h=1)
        p = 8 + j
        if j < t0:
            out.append((p, 0, 256, False, [(0, 256, "pad")]))
        elif j == t0:
            out.append((p, 0, 256, False, [(0, 128, "kill"), (128, 128, "pad")]))
        elif j == t1:
            out.append((p, 128, 128, False, [(128, 128, "kill")]))
    return out


def bias_columns():
    """Flat (p, kind) per bias-table column, in device emit order."""
    cols = []
    for g in range(NG):
        for p, qs, n, tri, segs in group_tiles(g):
            for cs, w, kind in segs:
                cols.append((p, kind))
    return cols


NI = len(bias_columns())


def _build_program():
    nc = bacc.Bacc("TRN2", target_bir_lowering=False, debug=False, num_devices=8)

    xT = nc.dram_tensor("xT", [E, S], BF16, kind="ExternalInput").ap()
    wq = nc.dram_tensor("wq", [128, EC * DH], BF16, kind="ExternalInput").ap()
    wk = nc.dram_tensor("wk", [128, EC * DH], BF16, kind="ExternalInput").ap()
    wv = nc.dram_tensor("wv", [128, EC * DH], BF16, kind="ExternalInput").ap()
    bq = nc.dram_tensor("bq", [128, 2], F32, kind="ExternalInput").ap()
    bk = nc.dram_tensor("bk", [128, 2], F32, kind="ExternalInput").ap()
    bvb = nc.dram_tensor("bvb", [128, DH], F32, kind="ExternalInput").ap()
    onesm = nc.dram_tensor("onesm", [128, 128], BF16, kind="ExternalInput").ap()
    ident = nc.dram_tensor("ident", [128, 128], BF16, kind="ExternalInput").ap()
    trim = nc.dram_tensor("trim", [128, 128], F32, kind="ExternalInput").ap()
    btab = nc.dram_tensor("btab", [128, NI], F32, kind="ExternalInput").ap()
    e0 = nc.dram_tensor("e0", [128, 8], F32R, kind="ExternalInput").ap()
    out = nc.dram_tensor("out", [SQ, DH], F32, kind="ExternalOutput").ap()

    with tile.TileContext(nc) as tc:
        _emit(nc, tc, xT, wq, wk, wv, bq, bk, bvb, onesm, ident, trim, btab,
              e0, out)
    nc.compile()
    return nc


def _emit(nc, tc, xT, wq, wk, wv, bq, bk, bvb, onesm, ident, trim, btab, e0,
          out):
    from contextlib import ExitStack

    with ExitStack() as ctx:
        const = ctx.enter_context(tc.tile_pool(name="const", bufs=1))
        persist = ctx.enter_context(tc.tile_pool(name="persist", bufs=1))

        # ---- persistent SBUF tensors -----------------------------------
        wq_sb = const.tile([128, EC, DH], BF16, tag="wq")
        wk_sb = const.tile([128, EC, DH], BF16, tag="wk")
        wv_sb = const.tile([128, EC, DH], BF16, tag="wv")
        # weights go on the ACT HWDGE ring (overlapping the xT loads on the
        # SP ring); wk first, in 4 chunks, so the first matmuls start early
        wk_r = wk.rearrange("p (c d) -> p c d", c=EC)
        for ep in range(4):
            nc.scalar.dma_start(wk_sb[:, ep * 4:(ep + 1) * 4, :],
                                wk_r[:, ep * 4:(ep + 1) * 4, :])
        wq_r = wq.rearrange("p (c d) -> p c d", c=EC)
        for ep in range(2):
            nc.scalar.dma_start(wq_sb[:, ep * 8:(ep + 1) * 8, :],
                                wq_r[:, ep * 8:(ep + 1) * 8, :])
        wv_r = wv.rearrange("p (c d) -> p c d", c=EC)
        for ep in range(2):
            nc.scalar.dma_start(wv_sb[:, ep * 8:(ep + 1) * 8, :],
                                wv_r[:, ep * 8:(ep + 1) * 8, :])

        # small constants on the gpsimd SWDGE ring: zero interference
        bq_sb = const.tile([128, 2], F32, tag="bq")
        bk_sb = const.tile([128, 2], F32, tag="bk")
        bvb_sb = const.tile([128, DH], F32, tag="bvb")
        onesm_sb = const.tile([128, 128], BF16, tag="onesm")
        ident_sb = const.tile([128, 128], BF16, tag="ident")
        trim_sb = const.tile([128, 128], F32, tag="trim")
        btab_sb = const.tile([128, NI], F32, tag="btab")
        e0_sb = const.tile([128, 8], F32R, tag="e0")
        nc.gpsimd.dma_start(bk_sb[:], bk[:])
        nc.gpsimd.dma_start(bq_sb[:], bq[:])
        nc.gpsimd.dma_start(btab_sb[:], btab[:])
        nc.gpsimd.dma_start(trim_sb[:], trim[:])
        nc.gpsimd.dma_start(onesm_sb[:], onesm[:])
        nc.gpsimd.dma_start(ident_sb[:], ident[:])
        nc.gpsimd.dma_start(e0_sb[:], e0[:])
        nc.gpsimd.dma_start(bvb_sb[:], bvb[:])

        kt_sb = persist.tile([128, 2, S], BF16, tag="ktp")
        qt_sb = persist.tile([128, 2, SQ], BF16, tag="qtp")
        v_sb = persist.tile([128, S // 128, DH], BF16, tag="vp")

        # ---- phase P: projections --------------------------------------
        xT_r = xT.rearrange("(c p) s -> p c s", p=128)
        with tc.tile_pool(name="xt", bufs=2) as xt_pool, \
             tc.tile_pool(name="proj_ps", bufs=2, space="PSUM") as proj_ps, \
             tc.tile_pool(name="v_ps", bufs=2, space="PSUM") as v_ps:
            # PE p-state warmup: ~4us of dummy matmuls on a memset tile so the
            # clock is fully ramped (2.4 GHz) when the first real matmul's
            # inputs land; runs entirely inside the DMA preamble shadow.
            warm = const.tile([128, 512], BF16, tag="warm")
            nc.gpsimd.memset(warm[:], 0.0)
            wps = proj_ps.tile([128, 512], F32, tag="warmps", bufs=1)
            for _ in range(8):
                nc.tensor.matmul(wps[:], warm[:, :128], warm[:], start=True,
                                 stop=True)
            for cg in range(NCG):
                xt = xt_pool.tile([128, EC, CG], BF16, tag="xt")
                npc = 4 if cg == 0 else 2
                w = EC // npc
                for ep in range(npc):
                    nc.sync.dma_start(
                        xt[:, ep * w:(ep + 1) * w, :],
                        xT_r[:, ep * w:(ep + 1) * w, cg * CG:(cg + 1) * CG],
                    )
                for dh2 in range(2):
                    ps = proj_ps.tile([128, CG], F32, tag="proj")
                    for e in range(EC):
                        nc.tensor.matmul(
                            ps[:],
                            wk_sb[:, e, dh2 * 128:(dh2 + 1) * 128],
                            xt[:, e, :],
                            start=(e == 0),
                            stop=(e == EC - 1),
                        )
                    nc.scalar.activation(
                        kt_sb[:, dh2, cg * CG:(cg + 1) * CG], ps[:],
                        AF.Identity, bias=bk_sb[:, dh2:dh2 + 1],
                    )
                if cg * CG < SQ:
                    for dh2 in range(2):
                        ps = proj_ps.tile([128, CG], F32, tag="proj")
                        for e in range(EC):
                            nc.tensor.matmul(
                                ps[:],
                                wq_sb[:, e, dh2 * 128:(dh2 + 1) * 128],
                                xt[:, e, :],
                                start=(e == 0),
                                stop=(e == EC - 1),
                            )
                        nc.scalar.activation(
                            qt_sb[:, dh2, cg * CG:(cg + 1) * CG], ps[:],
                            AF.Identity, bias=bq_sb[:, dh2:dh2 + 1],
                        )
                for kt4 in range(CG // 128):
                    kt = cg * (CG // 128) + kt4
                    psv = v_ps.tile([128, DH], F32, tag="vps")
                    for e in range(EC):
                        nc.tensor.matmul(
                            psv[:],
                            xt[:, e, kt4 * 128:(kt4 + 1) * 128],
                            wv_sb[:, e, :],
                            start=(e == 0),
                            stop=(e == EC - 1),
                        )
                    nc.vector.tensor_copy(v_sb[:, kt, :], psv[:])

        # ---- phase A: attention + per-group finalize -------------------
        bi = 0  # running bias-table column index (must match bias_columns())
        # PSUM budget: slots round up to whole banks, 8 total:
        # s_ps 3 + o_ps 2 (two tags) + d_ps 1 + f_ps 1 + r_ps 1 = 8
        with tc.tile_pool(name="s_ps", bufs=3, space="PSUM") as s_ps, \
             tc.tile_pool(name="o_ps", bufs=1, space="PSUM") as o_ps, \
             tc.tile_pool(name="d_ps", bufs=1, space="PSUM") as d_ps, \
             tc.tile_pool(name="f_ps", bufs=1, space="PSUM") as f_ps, \
             tc.tile_pool(name="r_ps", bufs=1, space="PSUM") as r_ps, \
             tc.tile_pool(name="pt", bufs=3) as pt_pool, \
             tc.tile_pool(name="fin", bufs=2) as fin:
            for g in range(NG):
                q0 = g * GQ
                tiles = group_tiles(g)
                otp = [
                    o_ps.tile([128, GQ], F32, tag=f"ot{d}", name=f"otp{g}_{d}")
                    for d in range(2)
                ]
                dnp = d_ps.tile([128, GQ], F32, tag="dn")
                last = len(tiles) - 1
                for ti, (p, qs, n, tri, segs) in enumerate(tiles):
                    sp = s_ps.tile([128, GQ], F32, tag="sp")
                    for dh2 in range(2):
                        nc.tensor.matmul(
                            sp[:, :n],
                            kt_sb[:, dh2, p * 128:(p + 1) * 128],
                            qt_sb[:, dh2, q0 + qs:q0 + qs + n],
                            start=(dh2 == 0),
                            stop=(dh2 == 1),
                        )
                    if tri:
                        nc.vector.tensor_tensor(
                            sp[:, :128], sp[:, :128], trim_sb[:], op=ALU.add
                        )
                    pt = pt_pool.tile([128, GQ], BF16, tag="pt")
                    for cs, w, kind in segs:
                        nc.scalar.activation(
                            pt[:, cs - qs:cs - qs + w],
                            sp[:, cs - qs:cs - qs + w],
                            AF.Exp, scale=SCALE, bias=btab_sb[:, bi:bi + 1],
                        )
                        bi += 1
                    for dh2 in range(2):
                        nc.tensor.matmul(
                            otp[dh2][:, qs:qs + n],
                            v_sb[:, p, dh2 * 128:(dh2 + 1) * 128],
                            pt[:, :n],
                            start=(ti == 0),
                            stop=(ti == last),
                        )
                    nc.tensor.matmul(
                        dnp[:, qs:qs + n],
                        onesm_sb[:],
                        pt[:, :n],
                        start=(ti == 0),
                        stop=(ti == last),
                    )

                # finalize this query group: recip + transpose + scale + store
                ot_g = fin.tile([128, 2, GQ], BF16, tag="otg")
                for dh2 in range(2):
                    nc.vector.tensor_copy(ot_g[:, dh2, :], otp[dh2][:])
                dn_g = fin.tile([128, GQ], F32R, tag="dng")
                nc.vector.tensor_copy(dn_g[:], dnp[:])
                rtp = r_ps.tile([128, 16], F32, tag="rt", name=f"rtp{g}")
                for qi in range(2):
                    nc.tensor.matmul(
                        rtp[:, qi * 8:(qi + 1) * 8],
                        dn_g[:, qi * 128:(qi + 1) * 128],
                        e0_sb[:],
                        start=(qi == 0),
                        stop=(qi == 1),
                    )
                rt_sb = fin.tile([128, 16], F32, tag="rtsb")
                nc.vector.tensor_copy(rt_sb[:], rtp[:])
                rc_sb = fin.tile([128, 16], F32, tag="rcsb")
                nc.vector.reciprocal(rc_sb[:], rt_sb[:])
                for qi in range(2):
                    qt = g * 2 + qi
                    ofp = f_ps.tile([128, DH], BF16, tag="of")
                    for dh2 in range(2):
                        nc.tensor.transpose(
                            ofp[:, dh2 * 128:(dh2 + 1) * 128],
                            ot_g[:, dh2, qi * 128:(qi + 1) * 128],
                            ident_sb[:],
                        )
                    ob = fin.tile([128, DH], F32, tag="ob")
                    # (ofp * 1/denom) + b_V fused on DVE, keeping the Scalar
                    # engine free for the next group's exps
                    nc.vector.scalar_tensor_tensor(
                        ob[:], ofp[:], rc_sb[:, qi * 8:qi * 8 + 1], bvb_sb[:],
                        op0=ALU.mult, op1=ALU.add,
                    )
                    nc.sync.dma_start(out[qt * 128:(qt + 1) * 128, :], ob[:])


def _get_program():
    global _prog_cache
    if _prog_cache is None:
        _prog_cache = _build_program()
    return _prog_cache


def kernel(x, causal_mask, padding_mask, W_Q, b_Q, W_K, b_K, W_V, b_V):
    global LAST_RESULTS
    from concourse.bass_utils import run_bass_kernel_spmd

    import ml_dtypes

    bf16 = ml_dtypes.bfloat16
    x = np.asarray(x, dtype=np.float32)
    pad = np.asarray(padding_mask)                   # [B, S]  True = masked key

    def tile_w(W):
        W = np.asarray(W, dtype=np.float32)
        return np.ascontiguousarray(
            W.reshape(EC, 128, DH).transpose(1, 0, 2).reshape(128, EC * DH)
        ).astype(bf16)

    W_Qb = tile_w(W_Q)
    W_Kb = tile_w(W_K)
    W_Vb = tile_w(W_V)
    bvb = np.ascontiguousarray(
        np.broadcast_to(np.asarray(b_V, dtype=np.float32), (128, DH))
    )
    bqh = np.ascontiguousarray(np.asarray(b_Q, dtype=np.float32).reshape(2, 128).T)
    bkh = np.ascontiguousarray(np.asarray(b_K, dtype=np.float32).reshape(2, 128).T)
    e0v = np.zeros((128, 8), dtype=np.float32)
    e0v[0, :] = 1.0
    onesm = np.ones((128, 128), dtype=bf16)
    identm = np.eye(128, dtype=np.float32).astype(bf16)
    q = np.arange(128)
    trimv = np.where(q[None, :] >= q[:, None], np.float32(0.0),
                     np.float32(NEG))  # [k, q]: attend iff q >= k
    trimv = np.ascontiguousarray(trimv)

    cols = bias_columns()
    in_maps = []
    for c in range(8):
        b, h = c // 2, c % 2
        qidx, kidx = core_key_order(h)
        xT = np.ascontiguousarray(x[b][kidx].T).astype(bf16)   # [E, S] permuted
        padp = pad[b][kidx].reshape(16, 128)                   # [tile, row]
        bt = np.zeros((128, len(cols)), dtype=np.float32)
        for i, (p, kind) in enumerate(cols):
            col = np.where(padp[p], np.float32(NEG), np.float32(0.0))
            if kind == "kill" and h == 0:
                col = np.full(128, np.float32(NEG))
            bt[:, i] = col
        in_maps.append({
            "xT": xT,
            "wq": W_Qb, "wk": W_Kb, "wv": W_Vb,
            "bq": bqh, "bk": bkh, "bvb": bvb,
            "onesm": onesm, "ident": identm, "trim": trimv,
            "btab": np.ascontiguousarray(bt), "e0": e0v,
        })

    nc = _get_program()
    res = run_bass_kernel_spmd(nc, in_maps, list(range(8)), trace=TRACE)
    LAST_RESULTS = res

    outp = np.empty((B, S, DH), dtype=np.float32)
    for c in range(8):
        b, h = c // 2, c % 2
        qidx, _ = core_key_order(h)
        outp[b][qidx] = res.results[c]["out"]
    return outp
